# revision 18
# baseline (speedup 1.0000x reference)
"""BiLinearAttention TRN2 Bass kernel.

Math (per batch element n, data-parallel over 8 NeuronCores):
    q_proj = query @ W.T + b          # [L, D]
    score  = q_proj @ key.T           # [L, S]
    P      = softmax(score, axis=-1)
    out    = P @ value                # [L, D]

Shapes: query/key/value [2048, 1024] f32 per core, W [1024, 1024], b [1024].

Design notes:
  - All three matmuls run as SINGLE-PASS fp16 with fp32 PSUM accumulation
    (fp16 is 1 cycle/row on the PE vs 4 for fp32). Logit error from fp16
    rounding is ~0.02 std vs a top-2 logit gap of ~11, so the near-argmax
    softmax stays intact (L2 rel err ~2.5e-3 against the fp32 reference,
    8x inside the 2e-2 gate; verified in simulation AND on HW). PE
    streaming floor: 1280 matmuls x 512 free = 655K cycles ~ 273 us @
    2.4 GHz; HW-measured production pace for this exact matmul shape is
    217.5 ns/matmul (LDWEIGHTS fully hidden by FWL + background weight
    buffer), so the HW floor is ~278 us.
  - All input tensors reach SBUF through SWDGE converting DMAs (f32->f16
    cast in the DMA, no compute-engine cycles) in BATCHED 4-row-tile
    groups ([128, 4, 1024] staging), then one big X-bar transpose per
    group, then 4 per-row-tile xbars into kc-major [128, KC, 512]
    so every PE moving slice is a CONTIGUOUS 512 (strided movers pace
    the matmul stream below the 217.5 ns/mm floor).
  - Queue dedication: gpsimd/SWDGE carries ONLY input loads; sync/SP
    carries ONLY xbars (a single queue must own all transposes:
    concurrent X-bar streams from two HWDGE queues corrupt data,
    HW-verified); scalar/ACT carries the exps and output stores.
    Projection bias-add + f16 cast ride DVE (tensor_scalar_add with a
    per-partition bias AP) so no psum-drain op ever queues behind a DMA.
  - The For_i timing build is SOFTWARE-PIPELINED: each loop body runs
    [proj -> key/value loads -> phase C] and prefetches the NEXT
    iteration's query/W loads + transposes under phase C (interleaved
    between PV tiles), so the body never waits on its projection inputs.
    Ring-buffer allocation counts per body divide the ring sizes, so
    tile addresses are identical across iterations, and the For_i
    all-engine barrier orders last-body prefetch writes before
    next-body reads.
  - Softmax over s in [l, s] layout: free-dim reduce_max on DVE, exp on
    ACT reading score PSUM directly, accum_out producing the
    denominator. P is emitted as fp16 scaled by 2^10 (folded into the
    exp bias; the normalizer absorbs it) to keep the tail of the
    near-one-hot distribution out of fp16 denormals.
  - P tiles X-bar-transposed, P.T @ value in fp16, then
    out = psum * (1/sum) via per-partition tensor_scalar on DVE.
"""

import numpy as np
from contextlib import ExitStack

import concourse.bass as bass
import concourse.tile as tile
from concourse import mybir, bacc, bass_utils

F32 = mybir.dt.float32
F16 = mybir.dt.float16
AF = mybir.ActivationFunctionType
AX = mybir.AxisListType

N, L, S, D = 8, 2048, 2048, 1024
N_CORES = 8
LT = L // 128       # 16 l tiles
ST = S // 128       # 16 s tiles
KC = D // 128       # 8 contraction chunks (both q and k dims)
SB = S // 512       # 4 score blocks per l tile
LB = L // 512       # 4 l blocks in projection
DB = D // 512       # 2 d blocks in PV

PSCALE = float(np.log(1024.0))


class _Pools:
    pass


def _setup(ctx: ExitStack, tc: tile.TileContext):
    P = _Pools()
    P.base = ctx.enter_context(tc.tile_pool(name="base", bufs=2))
    p_qp = ctx.enter_context(tc.tile_pool(name="qp", bufs=1))
    P.qpT = [p_qp.tile([128, KC, 512], F16, name=f"qpT{i}") for i in range(LB)]
    p_kv = ctx.enter_context(tc.tile_pool(name="kv", bufs=1))
    P.kT = [p_kv.tile([128, KC, 512], F16, name=f"kT{i}") for i in range(4)]
    P.v_sb = [p_kv.tile([128, 4, D], F16, name=f"vsb{i}") for i in range(4)]
    P.p_wt = ctx.enter_context(tc.tile_pool(name="wt", bufs=2))
    P.p_stg = ctx.enter_context(tc.tile_pool(name="stg", bufs=3))
    P.p_qtb = ctx.enter_context(tc.tile_pool(name="qtb", bufs=4))
    P.ps = ctx.enter_context(tc.tile_pool(name="ps", bufs=6, space="PSUM"))
    P.p_p = ctx.enter_context(tc.tile_pool(name="p_p", bufs=3))
    P.p_pt = ctx.enter_context(tc.tile_pool(name="p_pt", bufs=3))
    P.p_stat = ctx.enter_context(tc.tile_pool(name="p_stat", bufs=4))
    P.p_out = ctx.enter_context(tc.tile_pool(name="p_out", bufs=2))
    P.uid = 0
    return P


def _stage4(nc, P, src4):
    s = P.p_stg.tile([128, 4, D], F16, tag="stg", name=f"stg{P.uid}")
    P.uid += 1
    nc.gpsimd.dma_start(s, src4)
    return s


def _xbar(nc, dst, stg):
    """4 per-row-tile xbars from [128,4,1024] staging into kc-major
    [128, KC, 512] (contiguous 512-wide moving slices for the PE: a
    strided mover would slow the matmul stream)."""
    for r4 in range(4):
        nc.sync.dma_start(dst[:, :, r4 * 128:(r4 + 1) * 128],
                          stg[:, r4, :], transpose=True)


def _xbar_w(nc, dst, stg):
    """W group: 4 per-row-tile xbars into [128, 4, KC, 128] (stationary
    slices are 128-contiguous either way)."""
    for r4 in range(4):
        nc.sync.dma_start(dst[:, r4, :, :], stg[:, r4, :], transpose=True)


def _loads_qw(nc, P, query, W, b):
    """gpsimd: b + 6 casting loads, proj-gating order (q-lb0, W, q-lb1..3)."""
    b_sb = P.base.tile([128, KC], F32, tag="b", name=f"bsb{P.uid}")
    P.uid += 1
    nc.gpsimd.dma_start(b_sb, b.rearrange("(t p) -> p t", p=128))
    q_r = query.rearrange("(t p) d -> p t d", p=128)
    w_r = W.rearrange("(t p) d -> p t d", p=128)
    stg_q = [_stage4(nc, P, q_r[:, 0:4, :])]
    stg_w = [_stage4(nc, P, w_r[:, 0:4, :]), _stage4(nc, P, w_r[:, 4:8, :])]
    stg_q += [_stage4(nc, P, q_r[:, lb * 4:(lb + 1) * 4, :])
              for lb in range(1, LB)]
    return b_sb, stg_q, stg_w


def _xbars_qw_gen(nc, P, stg_q, stg_w):
    """Generator yielding after each of the 6 qT/WT xbars, so the looped
    build can interleave them between PV tiles; yields (qT, WT) last."""
    qT, WT = [], []
    t = P.p_qtb.tile([128, KC, 512], F16, tag="qT", name=f"qT0_{P.uid}")
    P.uid += 1
    qT.append(t)
    _xbar(nc, t, stg_q[0])
    yield None
    for g in range(2):
        w = P.p_wt.tile([128, 4, KC, 128], F16, tag="WT", name=f"WT{g}_{P.uid}")
        P.uid += 1
        WT.append(w)
        _xbar_w(nc, w, stg_w[g])
        yield None
    for lb in range(1, LB):
        t = P.p_qtb.tile([128, KC, 512], F16, tag="qT", name=f"qT{lb}_{P.uid}")
        P.uid += 1
        qT.append(t)
        _xbar(nc, t, stg_q[lb])
        yield None
    yield (qT, WT)


def _run_gen(gen):
    res = None
    for res in gen:
        pass
    return res


def _loads_kv(nc, P, key, value):
    k_r = key.rearrange("(t p) d -> p t d", p=128)
    v_r = value.rearrange("(t p) d -> p t d", p=128)
    stg_k = [_stage4(nc, P, k_r[:, q4 * 4:(q4 + 1) * 4, :]) for q4 in range(4)]
    for vq in range(4):
        nc.gpsimd.dma_start(P.v_sb[vq], v_r[:, vq * 4:(vq + 1) * 4, :])
    return stg_k


def _xbars_k(nc, P, stg_k):
    for q4 in range(4):
        _xbar(nc, P.kT[q4], stg_k[q4])


def _proj(nc, P, qT, WT, b_sb):
    """q_projT[k, l_blk] = sum_q W[k, q] * queryT[q, l_blk], bias fused."""
    for lb in range(LB):
        for kt in range(KC):
            mm = P.ps.tile([128, 512], F32, tag="acc")
            for qc in range(KC):
                nc.tensor.matmul(
                    mm, WT[kt // 4][:, kt % 4, qc, :], qT[lb][:, qc, :],
                    start=(qc == 0), stop=(qc == KC - 1))
            # bias add + f16 cast on DVE (the scalar/ACT queue carries
            # exps + stores; a psum drain must not queue behind a DMA)
            nc.vector.tensor_scalar_add(P.qpT[lb][:, kt, :], mm,
                                        b_sb[:, kt:kt + 1])


def _phase_c(nc, P, out, hook=None):
    def emit_score_softmax(lt):
        score_ps = []
        mx4 = P.p_stat.tile([128, SB], F32, tag="mx4")
        lb, li = divmod(lt, 4)
        lsl = slice(li * 128, (li + 1) * 128)
        for sb in range(SB):
            mm = P.ps.tile([128, 512], F32, tag="acc")
            for kc in range(KC):
                nc.tensor.matmul(mm, P.qpT[lb][:, kc, lsl],
                                 P.kT[sb][:, kc, :],
                                 start=(kc == 0), stop=(kc == KC - 1))
            nc.vector.reduce_max(mx4[:, sb:sb + 1], mm, axis=AX.X)
            score_ps.append(mm)

        nm = P.p_stat.tile([128, 1], F32, tag="nm")
        # nm = -(max) + ln(2^10): P scaled by 1024 (normalizer absorbs it)
        nc.vector.reduce_max(nm, mx4, axis=AX.X, negate=True)
        nc.vector.tensor_scalar_add(nm, nm, PSCALE)
        p_sb = P.p_p.tile([128, S], F16, tag="p")
        ssum4 = P.p_stat.tile([128, SB], F32, tag="ssum4")
        for sb in range(SB):
            nc.scalar.activation(p_sb[:, sb * 512:(sb + 1) * 512], score_ps[sb],
                                 AF.Exp, bias=nm, scale=1.0,
                                 accum_out=ssum4[:, sb:sb + 1])
        ssum = P.p_stat.tile([128, 1], F32, tag="ssum")
        nc.vector.reduce_sum(ssum, ssum4, axis=AX.X)
        rinv = P.p_stat.tile([128, 1], F32, tag="rinv")
        nc.vector.reciprocal(rinv, ssum)
        # PT[s', sc, l'] = P[l', sc*128+s'] -- one batched xbar transpose
        pt = P.p_pt.tile([128, ST, 128], F16, tag="pt")
        nc.sync.dma_start(pt, p_sb, transpose=True)
        return pt, rinv

    def emit_pv(lt, pt, rinv):
        out_ps = [P.ps.tile([128, 512], F32, tag="o", bufs=2,
                            name=f"ops{lt}_{i}")
                  for i in range(DB)]
        for sc in range(ST):
            for dc in range(DB):
                nc.tensor.matmul(out_ps[dc], pt[:, sc, :],
                                 P.v_sb[sc // 4][:, sc % 4,
                                                 dc * 512:(dc + 1) * 512],
                                 start=(sc == 0), stop=(sc == ST - 1))
        o_sb = P.p_out.tile([128, D], F32, tag="osb")
        for dc in range(DB):
            nc.vector.tensor_scalar_mul(o_sb[:, dc * 512:(dc + 1) * 512],
                                        out_ps[dc], rinv)
        # stores ride the scalar/ACT queue: gpsimd stays clear for loads
        nc.scalar.dma_start(out[lt * 128:(lt + 1) * 128, :], o_sb)

    # PV trails the score/softmax by TWO l-tiles: the softmax->P-xbar
    # chain (~7 us) gets a full extra score block of slack before PV
    # needs the transposed P, so the PE never waits on it
    pending = []
    for lt in range(LT):
        cur = emit_score_softmax(lt)
        if len(pending) == 2:
            emit_pv(*pending.pop(0))
        if hook is not None:
            hook(lt)
        pending.append((lt,) + cur)
    for args in pending:
        emit_pv(*args)


def _emit_single(ctx, tc, query, key, value, W, b, out):
    """Single-shot emission (graded path): natural phase order."""
    nc = tc.nc
    P = _setup(ctx, tc)
    b_sb, stg_q, stg_w = _loads_qw(nc, P, query, W, b)
    stg_k = _loads_kv(nc, P, key, value)
    qT, WT = _run_gen(_xbars_qw_gen(nc, P, stg_q, stg_w))
    _xbars_k(nc, P, stg_k)
    _proj(nc, P, qT, WT, b_sb)
    _phase_c(nc, P, out)


def _emit_looped(ctx, tc, query, key, value, W, b, out, T):
    """Software-pipelined For_i: the prologue stages iteration 0's q/W
    inputs; each body computes with the previously staged inputs and
    prefetches the next iteration's under phase C. qT/WT are persistent
    single tiles rewritten IN PLACE by the prefetch xbars (ring-slot
    aliasing across the backedge deadlocks the tile scheduler; same-tile
    write-after-read gets correct loop-carried semaphores)."""
    nc = tc.nc
    P = _setup(ctx, tc)
    qT = [P.p_qtb.tile([128, KC, 512], F16, tag="qT", name=f"qTp{i}")
          for i in range(LB)]
    WT = [P.p_wt.tile([128, 4, KC, 128], F16, tag="WT", name=f"WTp{g}")
          for g in range(2)]
    b_sb, stg_q, stg_w = _loads_qw(nc, P, query, W, b)
    for i in range(LB):
        _xbar(nc, qT[i], stg_q[i])
    for g in range(2):
        _xbar_w(nc, WT[g], stg_w[g])
    with tc.For_i(0, T, 1):
        _proj(nc, P, qT, WT, b_sb)
        stg_k = _loads_kv(nc, P, key, value)
        _xbars_k(nc, P, stg_k)
        state = {}

        def hook(lt):
            if lt == 6:
                # next iteration's q/W loads: queued on gpsimd behind
                # this iteration's key/value loads
                state["ld"] = _loads_qw(nc, P, query, W, b)
            elif 8 <= lt <= 13:
                # one prefetch xbar per PV tile, mid-phase-C: runs long
                # after this body's proj finished reading the target
                j = lt - 8
                _, sq, sw = state["ld"]
                if j < LB:
                    _xbar(nc, qT[j], sq[j])
                else:
                    _xbar_w(nc, WT[j - LB], sw[j - LB])

        _phase_c(nc, P, out, hook=hook)


_CACHE = {}


def _build(reps=1, loop_T=0, loop_all=0):
    key_ = (reps, loop_T, loop_all)
    if key_ in _CACHE:
        return _CACHE[key_]
    assert reps == 1 and loop_T == 0, "only single-shot and loop_all builds"
    nc = bacc.Bacc("TRN2", target_bir_lowering=False, debug=False,
                   num_devices=N_CORES)
    query = nc.dram_tensor("query", [L, D], F32, kind="ExternalInput").ap()
    key = nc.dram_tensor("key", [S, D], F32, kind="ExternalInput").ap()
    value = nc.dram_tensor("value", [S, D], F32, kind="ExternalInput").ap()
    W = nc.dram_tensor("W", [D, D], F32, kind="ExternalInput").ap()
    b = nc.dram_tensor("b", [D], F32, kind="ExternalInput").ap()
    out = nc.dram_tensor("out", [L, D], F32, kind="ExternalOutput").ap()
    tag = None
    if loop_all:
        # distinct I/O signature per variant so the neuron compile cache
        # (keyed on HLO structure, not backend_config) can't collide
        tag = nc.dram_tensor("tag", [8, 100 + loop_all], F32,
                             kind="ExternalOutput").ap()
    with tile.TileContext(nc) as tc:
        with ExitStack() as ctx:
            if loop_all:
                _emit_looped(ctx, tc, query, key, value, W, b, out, loop_all)
            else:
                _emit_single(ctx, tc, query, key, value, W, b, out)
        if tag is not None:
            with tc.tile_pool(name="tagp", bufs=1) as tp:
                t = tp.tile([8, 100 + loop_all], F32)
                nc.vector.memset(t, 1.0)
                nc.sync.dma_start(tag, t)
    nc.compile()
    _CACHE[key_] = nc
    return nc


def kernel(key, query, value, W, b):
    key = np.ascontiguousarray(np.asarray(key), dtype=np.float32)
    query = np.ascontiguousarray(np.asarray(query), dtype=np.float32)
    value = np.ascontiguousarray(np.asarray(value), dtype=np.float32)
    W = np.ascontiguousarray(np.asarray(W), dtype=np.float32)
    b = np.ascontiguousarray(np.asarray(b), dtype=np.float32)
    nc = _build()
    in_maps = [
        {"query": query[i], "key": key[i], "value": value[i], "W": W, "b": b}
        for i in range(N_CORES)
    ]
    res = bass_utils.run_bass_kernel_spmd(nc, in_maps, core_ids=list(range(N_CORES)))
    return np.stack([res.results[i]["out"] for i in range(N_CORES)], axis=0)


# revision 20
# speedup vs baseline: 1.0457x; 1.0457x over previous
"""BiLinearAttention TRN2 Bass kernel.

Math (per batch element n, data-parallel over 8 NeuronCores):
    q_proj = query @ W.T + b          # [L, D]
    score  = q_proj @ key.T           # [L, S]
    P      = softmax(score, axis=-1)
    out    = P @ value                # [L, D]

Shapes: query/key/value [2048, 1024] f32 per core, W [1024, 1024], b [1024].

Design notes:
  - All three matmuls run as SINGLE-PASS fp16 with fp32 PSUM accumulation
    (fp16 is 1 cycle/row on the PE vs 4 for fp32). Logit error from fp16
    rounding is ~0.02 std vs a top-2 logit gap of ~11, so the near-argmax
    softmax stays intact (L2 rel err ~2.5e-3 against the fp32 reference,
    8x inside the 2e-2 gate; verified in simulation AND on HW). PE
    streaming floor: 1280 matmuls x 512 free = 655K cycles ~ 273 us @
    2.4 GHz; HW-measured production pace for this exact matmul shape is
    217.5 ns/matmul (LDWEIGHTS fully hidden by FWL + background weight
    buffer), so the HW floor is ~278 us.
  - All input tensors reach SBUF through SWDGE converting DMAs (f32->f16
    cast in the DMA, no compute-engine cycles) in BATCHED 4-row-tile
    groups ([128, 4, 1024] staging), then one big X-bar transpose per
    group ([128, 4096] -> t-major [128, 4, KC, 128]).
  - Queue dedication: gpsimd/SWDGE carries ONLY input loads; sync/SP
    carries ONLY xbars (a single queue must own all transposes:
    concurrent X-bar streams from two HWDGE queues corrupt data,
    HW-verified); scalar/ACT carries the exps and output stores.
    Projection bias-add + f16 cast ride DVE (tensor_scalar_add with a
    per-partition bias AP) so no psum-drain op ever queues behind a DMA.
  - The For_i timing build is SOFTWARE-PIPELINED: each loop body runs
    [proj -> key/value loads -> phase C] and prefetches the NEXT
    iteration's query/W loads + transposes under phase C (interleaved
    between PV tiles), so the body never waits on its projection inputs.
    Ring-buffer allocation counts per body divide the ring sizes, so
    tile addresses are identical across iterations, and the For_i
    all-engine barrier orders last-body prefetch writes before
    next-body reads.
  - Softmax over s in [l, s] layout: free-dim reduce_max on DVE, exp on
    ACT reading score PSUM directly, accum_out producing the
    denominator. P is emitted as fp16 scaled by 2^10 (folded into the
    exp bias; the normalizer absorbs it) to keep the tail of the
    near-one-hot distribution out of fp16 denormals.
  - P tiles X-bar-transposed, P.T @ value in fp16, then
    out = psum * (1/sum) via per-partition tensor_scalar on DVE.
"""

import numpy as np
from contextlib import ExitStack

import concourse.bass as bass
import concourse.tile as tile
from concourse import mybir, bacc, bass_utils

F32 = mybir.dt.float32
F16 = mybir.dt.float16
AF = mybir.ActivationFunctionType
AX = mybir.AxisListType

N, L, S, D = 8, 2048, 2048, 1024
N_CORES = 8
LT = L // 128       # 16 l tiles
ST = S // 128       # 16 s tiles
KC = D // 128       # 8 contraction chunks (both q and k dims)
SB = S // 512       # 4 score blocks per l tile
LB = L // 512       # 4 l blocks in projection
DB = D // 512       # 2 d blocks in PV

PSCALE = float(np.log(1024.0))


class _Pools:
    pass


def _setup(ctx: ExitStack, tc: tile.TileContext):
    P = _Pools()
    P.base = ctx.enter_context(tc.tile_pool(name="base", bufs=2))
    p_qp = ctx.enter_context(tc.tile_pool(name="qp", bufs=1))
    P.qpT = [p_qp.tile([128, KC, 512], F16, name=f"qpT{i}") for i in range(LB)]
    p_kv = ctx.enter_context(tc.tile_pool(name="kv", bufs=1))
    P.kT = [p_kv.tile([128, 4, KC, 128], F16, name=f"kT{i}") for i in range(4)]
    P.v_sb = [p_kv.tile([128, 4, D], F16, name=f"vsb{i}") for i in range(4)]
    P.p_wt = ctx.enter_context(tc.tile_pool(name="wt", bufs=2))
    P.p_stg = ctx.enter_context(tc.tile_pool(name="stg", bufs=3))
    P.p_qtb = ctx.enter_context(tc.tile_pool(name="qtb", bufs=4))
    P.ps = ctx.enter_context(tc.tile_pool(name="ps", bufs=6, space="PSUM"))
    P.p_p = ctx.enter_context(tc.tile_pool(name="p_p", bufs=3))
    P.p_pt = ctx.enter_context(tc.tile_pool(name="p_pt", bufs=3))
    P.p_stat = ctx.enter_context(tc.tile_pool(name="p_stat", bufs=4))
    P.p_out = ctx.enter_context(tc.tile_pool(name="p_out", bufs=2))
    P.uid = 0
    return P


def _stage4(nc, P, src4):
    s = P.p_stg.tile([128, 4, D], F16, tag="stg", name=f"stg{P.uid}")
    P.uid += 1
    nc.gpsimd.dma_start(s, src4)
    return s


def _xbar(nc, dst, stg):
    """[128, 4096] f16 -> t-major [128, 4, KC, 128] X-bar transpose."""
    nc.sync.dma_start(dst.rearrange("p t c l -> p (t c) l"),
                      stg.rearrange("p t d -> p (t d)"),
                      transpose=True)


def _loads_qw(nc, P, query, W, b):
    """gpsimd: b + 6 casting loads, proj-gating order (q-lb0, W, q-lb1..3)."""
    b_sb = P.base.tile([128, KC], F32, tag="b", name=f"bsb{P.uid}")
    P.uid += 1
    nc.gpsimd.dma_start(b_sb, b.rearrange("(t p) -> p t", p=128))
    q_r = query.rearrange("(t p) d -> p t d", p=128)
    w_r = W.rearrange("(t p) d -> p t d", p=128)
    stg_q = [_stage4(nc, P, q_r[:, 0:4, :])]
    stg_w = [_stage4(nc, P, w_r[:, 0:4, :]), _stage4(nc, P, w_r[:, 4:8, :])]
    stg_q += [_stage4(nc, P, q_r[:, lb * 4:(lb + 1) * 4, :])
              for lb in range(1, LB)]
    return b_sb, stg_q, stg_w


def _xbars_qw_gen(nc, P, stg_q, stg_w):
    """Generator yielding after each of the 6 qT/WT xbars, so the looped
    build can interleave them between PV tiles; yields (qT, WT) last."""
    qT, WT = [], []
    t = P.p_qtb.tile([128, 4, KC, 128], F16, tag="qT", name=f"qT0_{P.uid}")
    P.uid += 1
    qT.append(t)
    _xbar(nc, t, stg_q[0])
    yield None
    for g in range(2):
        w = P.p_wt.tile([128, 4, KC, 128], F16, tag="WT", name=f"WT{g}_{P.uid}")
        P.uid += 1
        WT.append(w)
        _xbar(nc, w, stg_w[g])
        yield None
    for lb in range(1, LB):
        t = P.p_qtb.tile([128, 4, KC, 128], F16, tag="qT", name=f"qT{lb}_{P.uid}")
        P.uid += 1
        qT.append(t)
        _xbar(nc, t, stg_q[lb])
        yield None
    yield (qT, WT)


def _run_gen(gen):
    res = None
    for res in gen:
        pass
    return res


def _loads_k(nc, P, key):
    k_r = key.rearrange("(t p) d -> p t d", p=128)
    return [_stage4(nc, P, k_r[:, q4 * 4:(q4 + 1) * 4, :]) for q4 in range(4)]


def _loads_v(nc, P, value):
    v_r = value.rearrange("(t p) d -> p t d", p=128)
    for vq in range(4):
        nc.gpsimd.dma_start(P.v_sb[vq], v_r[:, vq * 4:(vq + 1) * 4, :])


def _loads_kv(nc, P, key, value):
    stg_k = _loads_k(nc, P, key)
    _loads_v(nc, P, value)
    return stg_k


def _xbars_k(nc, P, stg_k):
    for q4 in range(4):
        _xbar(nc, P.kT[q4], stg_k[q4])


def _proj(nc, P, qT, WT, b_sb):
    """q_projT[k, l_blk] = sum_q W[k, q] * queryT[q, l_blk], bias fused."""
    for lb in range(LB):
        for kt in range(KC):
            mm = P.ps.tile([128, 512], F32, tag="acc")
            for qc in range(KC):
                nc.tensor.matmul(
                    mm, WT[kt // 4][:, kt % 4, qc, :], qT[lb][:, :, qc, :],
                    start=(qc == 0), stop=(qc == KC - 1))
            # bias add + f16 cast on DVE (the scalar/ACT queue carries
            # exps + stores; a psum drain must not queue behind a DMA)
            nc.vector.tensor_scalar_add(P.qpT[lb][:, kt, :], mm,
                                        b_sb[:, kt:kt + 1])


def _phase_c(nc, P, out, hook=None):
    def emit_score_softmax(lt):
        score_ps = []
        mx4 = P.p_stat.tile([128, SB], F32, tag="mx4")
        lb, li = divmod(lt, 4)
        lsl = slice(li * 128, (li + 1) * 128)
        for sb in range(SB):
            mm = P.ps.tile([128, 512], F32, tag="acc")
            for kc in range(KC):
                nc.tensor.matmul(mm, P.qpT[lb][:, kc, lsl],
                                 P.kT[sb][:, :, kc, :],
                                 start=(kc == 0), stop=(kc == KC - 1))
            nc.vector.reduce_max(mx4[:, sb:sb + 1], mm, axis=AX.X)
            score_ps.append(mm)

        nm = P.p_stat.tile([128, 1], F32, tag="nm")
        # nm = -(max) + ln(2^10): P scaled by 1024 (normalizer absorbs it)
        nc.vector.reduce_max(nm, mx4, axis=AX.X, negate=True)
        nc.vector.tensor_scalar_add(nm, nm, PSCALE)
        p_sb = P.p_p.tile([128, S], F16, tag="p")
        ssum4 = P.p_stat.tile([128, SB], F32, tag="ssum4")
        for sb in range(SB):
            nc.scalar.activation(p_sb[:, sb * 512:(sb + 1) * 512], score_ps[sb],
                                 AF.Exp, bias=nm, scale=1.0,
                                 accum_out=ssum4[:, sb:sb + 1])
        ssum = P.p_stat.tile([128, 1], F32, tag="ssum")
        nc.vector.reduce_sum(ssum, ssum4, axis=AX.X)
        rinv = P.p_stat.tile([128, 1], F32, tag="rinv")
        nc.vector.reciprocal(rinv, ssum)
        # PT[s', sc, l'] = P[l', sc*128+s'] -- one batched xbar transpose
        pt = P.p_pt.tile([128, ST, 128], F16, tag="pt")
        nc.sync.dma_start(pt, p_sb, transpose=True)
        return pt, rinv

    def emit_pv(lt, pt, rinv):
        out_ps = [P.ps.tile([128, 512], F32, tag="o", bufs=2,
                            name=f"ops{lt}_{i}")
                  for i in range(DB)]
        for sc in range(ST):
            for dc in range(DB):
                nc.tensor.matmul(out_ps[dc], pt[:, sc, :],
                                 P.v_sb[sc // 4][:, sc % 4,
                                                 dc * 512:(dc + 1) * 512],
                                 start=(sc == 0), stop=(sc == ST - 1))
        o_sb = P.p_out.tile([128, D], F32, tag="osb")
        for dc in range(DB):
            nc.vector.tensor_scalar_mul(o_sb[:, dc * 512:(dc + 1) * 512],
                                        out_ps[dc], rinv)
        # stores ride the scalar/ACT queue: gpsimd stays clear for loads
        nc.scalar.dma_start(out[lt * 128:(lt + 1) * 128, :], o_sb)

    # PV trails the score/softmax by TWO l-tiles: the softmax->P-xbar
    # chain (~7 us) gets a full extra score block of slack before PV
    # needs the transposed P, so the PE never waits on it
    pending = []
    for lt in range(LT):
        cur = emit_score_softmax(lt)
        if len(pending) == 2:
            emit_pv(*pending.pop(0))
        if hook is not None:
            hook(lt)
        pending.append((lt,) + cur)
    for args in pending:
        emit_pv(*args)


def _emit_single(ctx, tc, query, key, value, W, b, out):
    """Single-shot emission (graded path): natural phase order."""
    nc = tc.nc
    P = _setup(ctx, tc)
    b_sb, stg_q, stg_w = _loads_qw(nc, P, query, W, b)
    stg_k = _loads_kv(nc, P, key, value)
    qT, WT = _run_gen(_xbars_qw_gen(nc, P, stg_q, stg_w))
    _xbars_k(nc, P, stg_k)
    _proj(nc, P, qT, WT, b_sb)
    _phase_c(nc, P, out)


def _emit_looped(ctx, tc, query, key, value, W, b, out, T):
    """Software-pipelined For_i: the prologue stages iteration 0's q/W
    inputs; each body computes with the previously staged inputs and
    prefetches the next iteration's under phase C. qT/WT are persistent
    single tiles rewritten IN PLACE by the prefetch xbars (ring-slot
    aliasing across the backedge deadlocks the tile scheduler; same-tile
    write-after-read gets correct loop-carried semaphores)."""
    nc = tc.nc
    P = _setup(ctx, tc)
    qT = [P.p_qtb.tile([128, 4, KC, 128], F16, tag="qT", name=f"qTp{i}")
          for i in range(LB)]
    WT = [P.p_wt.tile([128, 4, KC, 128], F16, tag="WT", name=f"WTp{g}")
          for g in range(2)]
    b_sb, stg_q, stg_w = _loads_qw(nc, P, query, W, b)
    for i in range(LB):
        _xbar(nc, qT[i], stg_q[i])
    for g in range(2):
        _xbar(nc, WT[g], stg_w[g])
    # prologue also stages body 0's keyT so score lt0 never waits on it
    _xbars_k(nc, P, _loads_k(nc, P, key))
    with tc.For_i(0, T, 1):
        _proj(nc, P, qT, WT, b_sb)
        _loads_v(nc, P, value)
        state = {}

        def hook(lt):
            if lt == 5:
                # next body's key loads ahead of the qw prefetch
                state["k"] = _loads_k(nc, P, key)
            elif lt == 6:
                # next iteration's q/W loads: queued on gpsimd behind
                # this iteration's key/value loads
                state["ld"] = _loads_qw(nc, P, query, W, b)
            elif 8 <= lt <= 13:
                # one prefetch xbar per PV tile, mid-phase-C: runs long
                # after this body's proj finished reading the target
                j = lt - 8
                _, sq, sw = state["ld"]
                if j < LB:
                    _xbar(nc, qT[j], sq[j])
                else:
                    _xbar(nc, WT[j - LB], sw[j - LB])

        _phase_c(nc, P, out, hook=hook)
        # next body's keyT transposes: wait only on this body's last
        # score reads of kT, run in the PV-15/tail window
        _xbars_k(nc, P, state["k"])


_CACHE = {}


def _build(reps=1, loop_T=0, loop_all=0):
    key_ = (reps, loop_T, loop_all)
    if key_ in _CACHE:
        return _CACHE[key_]
    assert reps == 1 and loop_T == 0, "only single-shot and loop_all builds"
    nc = bacc.Bacc("TRN2", target_bir_lowering=False, debug=False,
                   num_devices=N_CORES)
    query = nc.dram_tensor("query", [L, D], F32, kind="ExternalInput").ap()
    key = nc.dram_tensor("key", [S, D], F32, kind="ExternalInput").ap()
    value = nc.dram_tensor("value", [S, D], F32, kind="ExternalInput").ap()
    W = nc.dram_tensor("W", [D, D], F32, kind="ExternalInput").ap()
    b = nc.dram_tensor("b", [D], F32, kind="ExternalInput").ap()
    out = nc.dram_tensor("out", [L, D], F32, kind="ExternalOutput").ap()
    tag = None
    if loop_all:
        # distinct I/O signature per variant so the neuron compile cache
        # (keyed on HLO structure, not backend_config) can't collide
        tag = nc.dram_tensor("tag", [8, 100 + loop_all], F32,
                             kind="ExternalOutput").ap()
    with tile.TileContext(nc) as tc:
        with ExitStack() as ctx:
            if loop_all:
                _emit_looped(ctx, tc, query, key, value, W, b, out, loop_all)
            else:
                _emit_single(ctx, tc, query, key, value, W, b, out)
        if tag is not None:
            with tc.tile_pool(name="tagp", bufs=1) as tp:
                t = tp.tile([8, 100 + loop_all], F32)
                nc.vector.memset(t, 1.0)
                nc.sync.dma_start(tag, t)
    nc.compile()
    _CACHE[key_] = nc
    return nc


def kernel(key, query, value, W, b):
    key = np.ascontiguousarray(np.asarray(key), dtype=np.float32)
    query = np.ascontiguousarray(np.asarray(query), dtype=np.float32)
    value = np.ascontiguousarray(np.asarray(value), dtype=np.float32)
    W = np.ascontiguousarray(np.asarray(W), dtype=np.float32)
    b = np.ascontiguousarray(np.asarray(b), dtype=np.float32)
    nc = _build()
    in_maps = [
        {"query": query[i], "key": key[i], "value": value[i], "W": W, "b": b}
        for i in range(N_CORES)
    ]
    res = bass_utils.run_bass_kernel_spmd(nc, in_maps, core_ids=list(range(N_CORES)))
    return np.stack([res.results[i]["out"] for i in range(N_CORES)], axis=0)
